# revision 10
# baseline (speedup 1.0000x reference)
"""Distributed brute-force KNN (retrieval) kernel for 8 Trainium2 NeuronCores.

Strategy
--------
Candidates are sharded row-wise across the 8 cores (125k each). Each core
computes quantized scores for all 512 queries against its shard with the
tensor engine (bf16, exact integer-grid arithmetic). Index-embedding rows
add a tiny per-column offset u = iw * 2^-18 (iw = position within the
1024-wide half-window), so the fp32 PSUM value carries BOTH the score and
the 10 index bits exactly (score grid 1/128, |s| < 64 whp).

PSUM is drained by the two engines that can read it, load-balanced at
half-window (1024 col) granularity:

  * DVE max8 halves: top-8 (value+index packed) per query per half-window.
    Host decodes indices and keeps decoded scores > t_q.
  * ACT relu+accum halves: activation(Relu, bias=-t_q) with accumulate.
    The accumulated value IS the packed survivor when the half held
    exactly one gated candidate; exact-fp32 verification rejects every
    other case, and those halves are recovered by an exact host rescan.

Thresholds t_q are picked host-side from a random sample so that ~50
candidates per core per query survive (a guaranteed superset of the
global top-k). Host rescores all survivors exactly (float64) and merges
to the global top-k.
"""

import numpy as np
import ml_dtypes

B, D, N = 512, 64, 1_000_000
NCORES = 8
NSHARD = N // NCORES            # 125000
HALF = 1024                     # drain granularity (cols per engine instr)
NH_FULL = NSHARD // HALF        # 122 full halves
RUNT = NSHARD - NH_FULL * HALF  # 72 leftover candidates
RUNT_FD = 72                    # runt drained at exact width
NPAD = NH_FULL * HALF + RUNT_FD  # 125056
KAUG = 66                       # 64 dims + u_hi + u_lo
QB = B // 128                   # 4 query blocks

SAMP = 32768                    # host-side sample size per core
RSTAR = 14                      # threshold = (RSTAR-th largest sample) - 1/128

_Q_GRID = 8.0                   # queries quantized to 1/8
_C_GRID = 16.0                  # candidates quantized to 1/16  -> score grid 1/128
_EMB = 2.0 ** 18                # index embedding unit 2^-18 (10 bits per half)

# Half-window engine assignment: even full halves -> DVE max8, odd -> ACT
# relu+accum, runt -> DVE. Gives 62 DVE / 61 ACT per query block, balancing
# the two engines' per-instruction costs (~1192ns vs ~1184ns).
D_HALVES = [h for h in range(NH_FULL) if h % 2 == 0]
A_HALVES = [h for h in range(NH_FULL) if h % 2 == 1] + [NH_FULL]
_D_RANK = {h: i for i, h in enumerate(D_HALVES)}
_A_RANK = {h: i for i, h in enumerate(A_HALVES)}
ND = len(D_HALVES)              # 62 (includes runt)
NA = len(A_HALVES)              # 61

_CACHE = {}


def _half_off(h):
    return h * HALF


def _half_fd(h):
    return HALF if h < NH_FULL else RUNT_FD


def _build_bass():
    import concourse.tile as tile
    import concourse.mybir as mybir
    from concourse import bacc

    nc = bacc.Bacc("TRN2", target_bir_lowering=False, debug=False,
                   enable_asserts=False)
    q_dram = nc.dram_tensor("qaug", (KAUG, B), mybir.dt.bfloat16,
                            kind="ExternalInput")
    c_dram = nc.dram_tensor("caug", (KAUG, NPAD), mybir.dt.bfloat16,
                            kind="ExternalInput")
    nt_dram = nc.dram_tensor("negt", (128, QB), mybir.dt.float32,
                             kind="ExternalInput")
    out_dram = nc.dram_tensor("out_vals", (B, ND * 8), mybir.dt.float32,
                              kind="ExternalOutput")
    acc_dram = nc.dram_tensor("out_acc", (B, NA), mybir.dt.float32,
                              kind="ExternalOutput")

    with tile.TileContext(nc) as tc:
        with tc.tile_pool(name="cts", bufs=6) as ctp, \
             tc.tile_pool(name="qp", bufs=1) as qp, \
             tc.tile_pool(name="outp", bufs=1) as outp, \
             tc.tile_pool(name="psD", bufs=2, space="PSUM") as psD, \
             tc.tile_pool(name="psA", bufs=2, space="PSUM") as psA:

            qt = qp.tile([KAUG, B], mybir.dt.bfloat16)
            nc.gpsimd.dma_start(qt[:], q_dram.ap()[:, :])
            nt = qp.tile([128, QB], mybir.dt.float32)
            nc.gpsimd.dma_start(nt[:], nt_dram.ap()[:, :])

            ov = [outp.tile([128, ND * 8], mybir.dt.float32, tag=f"ov{qb}",
                            name=f"ov{qb}")
                  for qb in range(QB)]
            oa = [outp.tile([128, NA], mybir.dt.float32, tag=f"oa{qb}",
                            name=f"oa{qb}")
                  for qb in range(QB)]

            for h in [NH_FULL] + list(range(NH_FULL)):
                fd = _half_fd(h)
                off = _half_off(h)
                ct = ctp.tile([KAUG, HALF], mybir.dt.bfloat16, tag="ct")
                nc.gpsimd.dma_start(ct[:, 0:fd], c_dram.ap()[:, off:off + fd])
                is_d = h in _D_RANK
                for qb in range(QB):
                    if is_d:
                        pt = psD.tile([128, HALF], mybir.dt.float32,
                                      tag="ptD", name="ptD")
                    else:
                        pt = psA.tile([128, HALF], mybir.dt.float32,
                                      tag="ptA", name="ptA")
                    for s in range(0, fd, 512):
                        w = min(512, fd - s)
                        nc.tensor.matmul(pt[:, s:s + w],
                                         qt[:, qb * 128:(qb + 1) * 128],
                                         ct[:, s:s + w],
                                         start=True, stop=True)
                    if is_d:
                        o = _D_RANK[h] * 8
                        nc.vector.max(ov[qb][:, o:o + 8], pt[:, 0:fd])
                    else:
                        a = _A_RANK[h]
                        nc.scalar.activation(
                            pt[:, 0:fd], pt[:, 0:fd],
                            mybir.ActivationFunctionType.Relu,
                            bias=nt[:, qb:qb + 1],
                            accum_out=oa[qb][:, a:a + 1])

            # chunked output DMAs on the otherwise-idle SP HWDGE queue; the
            # tile scheduler hoists each chunk to right after its last writer
            for qb in range(QB):
                rows = slice(qb * 128, (qb + 1) * 128)
                for arr, dram, n in ((ov[qb], out_dram, ND * 8),
                                     (oa[qb], acc_dram, NA)):
                    bnds = [n * i // 4 for i in range(5)]
                    for lo, hi in zip(bnds, bnds[1:]):
                        if hi > lo:
                            nc.sync.dma_start(dram.ap()[rows, lo:hi],
                                              arr[:, lo:hi])
    nc.compile()
    return nc


def _get_nc():
    if "nc" not in _CACHE:
        _CACHE["nc"] = _build_bass()
    return _CACHE["nc"]


def _bf16(a):
    """Exact fp32->bf16 for values already representable in bf16 (bit shift —
    much faster than ml_dtypes astype; truncation == rounding here)."""
    return (np.ascontiguousarray(a, np.float32).view(np.uint32) >> 16) \
        .astype(np.uint16).view(ml_dtypes.bfloat16)


def _u_of(iw):
    """Exact embedding offset u(iw) for iw in [0, 1024), float64-exact."""
    iw = np.asarray(iw, np.int64)
    return ((iw >> 6).astype(np.float64) * (2.0 ** -12)
            + (iw & 63).astype(np.float64) * (2.0 ** -18))


def _prep_inputs(queries, candidates):
    """Host-side staging: quantize, sample thresholds, build augmented operands."""
    qq = np.round(queries.astype(np.float32) * _Q_GRID) / _Q_GRID
    cc = np.round(candidates.astype(np.float32) * _C_GRID) / _C_GRID

    rng = np.random.default_rng(0x5EED)
    iw = np.arange(NPAD, dtype=np.int64) % HALF
    u_hi = ((iw >> 6).astype(np.float32)) * (2.0 ** -12)   # 4 bits, bf16-exact
    u_lo = ((iw & 63).astype(np.float32)) * (2.0 ** -18)   # 6 bits, bf16-exact

    in_maps = []
    t_all = np.zeros((NCORES, B), np.float32)
    for c in range(NCORES):
        shard = cc[c * NSHARD:(c + 1) * NSHARD]            # [125000, 64]
        sidx = rng.choice(NSHARD, SAMP, replace=False)
        s_samp = qq @ shard[sidx].T                        # [512, SAMP] exact fp32
        t_raw = np.partition(s_samp, SAMP - RSTAR, axis=1)[:, SAMP - RSTAR]
        t = (t_raw - np.float32(1.0 / 128.0)).astype(np.float32)
        t_all[c] = t                                       # on grid, strictly below

        qaug = np.zeros((KAUG, B), np.float32)
        qaug[:D] = qq.T
        qaug[D] = 1.0
        qaug[D + 1] = 1.0

        caug = np.zeros((KAUG, NPAD), np.float32)
        caug[:D, :NSHARD] = shard.T
        caug[D] = u_hi
        caug[D + 1] = u_lo

        negt = np.zeros((128, QB), np.float32)
        for qb in range(QB):
            negt[:, qb] = -t[qb * 128:(qb + 1) * 128]

        in_maps.append({"qaug": _bf16(qaug), "caug": _bf16(caug),
                        "negt": negt})
    return in_maps, qq, cc, t_all


def _decode_and_merge(queries, candidates, core_outs, qq, cc, t_all, k):
    """Decode embedded indices, rescore survivors exactly, global top-k.

    DVE max8 halves: top-8 packed values v = s~ + u per half; decode u to get
    the index, keep decoded s~ > t. If the 8th slot also decodes above t the
    half may hold >8 survivors -> exact host rescan. ACT halves: accumulated
    relu(s~ + u - t); a single gated candidate is recovered exactly (verified
    bitwise), anything else -> exact host rescan on the quantized grid.
    """
    d_off = np.array([_half_off(h) for h in D_HALVES], np.int64)
    a_off = np.array([_half_off(h) for h in A_HALVES], np.int64)
    qn, cidx_all = [], []
    rescan = []                                            # (core, q, off, fd, kind)
    for c, (ov, oa) in enumerate(core_outs):
        t64 = t_all[c].astype(np.float64)
        # --- DVE max8 halves: 8 slots per half ---
        v = np.asarray(ov, np.float64)                     # [512, ND*8]
        m = np.rint(v * _EMB).astype(np.int64)
        iwv = m % HALF
        s_dec = v - _u_of(iwv)                             # exact: v on 2^-18 grid
        cand = d_off[np.arange(ND * 8)[None, :] // 8] + iwv
        ok = (s_dec > t64[:, None]) & (cand < NSHARD) & (np.abs(v) < 64.0)
        qi, slot = np.nonzero(ok)
        qn.append(qi)
        cidx_all.append(cand[qi, slot] + c * NSHARD)
        # overflow guards: 8th slot above threshold, or |v| >= 64 (cannot
        # decode exactly) -> rescan the half
        odd = ok[:, 7::8] | ((np.abs(v) >= 64.0)[:, 7::8])
        for q, hd in zip(*np.nonzero(odd)):
            rescan.append((c, q, int(d_off[hd]), _half_fd(D_HALVES[hd]), "D"))
        big = (np.abs(v) >= 64.0) & (v != 0.0)
        for q, slot_ in zip(*np.nonzero(big[:, :])):
            rescan.append((c, q, int(d_off[slot_ // 8]),
                           _half_fd(D_HALVES[slot_ // 8]), "D"))
        # --- ACT accum halves ---
        S = np.asarray(oa, np.float64)                     # [512, NA]
        qi2, col = np.nonzero(S > 0)
        x = S[qi2, col] + t64[qi2]                         # = s~ + u (if single)
        m2 = np.rint(x * _EMB).astype(np.int64)
        iw2 = m2 % HALF
        cand2 = a_off[col] + iw2
        inb = (cand2 < NSHARD) & (np.abs(x) < 64.0)
        vc = np.full(x.shape, np.nan, np.float64)
        if inb.any():
            s_ex = np.einsum("md,md->m",
                             qq[qi2[inb]].astype(np.float64),
                             cc[c * NSHARD + cand2[inb]].astype(np.float64))
            vc[inb] = s_ex + _u_of(iw2[inb]) - t64[qi2[inb]]
        good = inb & (vc == (S[qi2, col]))
        qn.append(qi2[good])
        cidx_all.append(cand2[good] + c * NSHARD)
        bad = ~good
        for q, cl in zip(qi2[bad], col[bad]):
            rescan.append((c, q, int(a_off[cl]), HALF, "A"))
    # --- rescan unresolved halves with exact grid arithmetic ---
    if rescan:
        from collections import defaultdict
        groups = defaultdict(list)
        for c, q, off, fd, kind in rescan:
            groups[(c, off, fd, kind)].append(q)
        for (c, off, fd, kind), qs in groups.items():
            qs = np.unique(np.array(qs))
            lo = c * NSHARD + off
            hi = min(lo + fd, (c + 1) * NSHARD)
            if hi <= lo:
                continue
            s_blk = qq[qs].astype(np.float64) @ cc[lo:hi].astype(np.float64).T
            if kind == "A":
                u_blk = _u_of(np.arange(hi - lo))
                hit = (s_blk + u_blk[None, :]) > t_all[c][qs, None]
            else:
                hit = s_blk > t_all[c][qs, None]
            r, cnd = np.nonzero(hit)
            qn.append(qs[r])
            cidx_all.append(lo + cnd)
    qi = np.concatenate(qn)
    ci = np.concatenate(cidx_all)

    # dedupe (rescans can re-report decoded survivors)
    key = qi.astype(np.int64) * N + ci
    _, uniq = np.unique(key, return_index=True)
    qi, ci = qi[uniq], ci[uniq]

    # exact rescore of survivors in float64, then order like jax.lax.top_k
    qf = queries.astype(np.float64)
    cf = candidates.astype(np.float64)
    vals = np.einsum("md,md->m", qf[qi], cf[ci])
    vals32 = vals.astype(np.float32)

    order = np.lexsort((ci, -vals, qi))
    qi, ci, vals32 = qi[order], ci[order], vals32[order]
    counts = np.bincount(qi, minlength=B)

    out_v = np.zeros((B, k), np.float32)
    out_i = np.zeros((B, k), np.int32)
    starts = np.concatenate(([0], np.cumsum(counts)))
    for b in range(B):
        s, e = starts[b], starts[b + 1]
        if e - s < k:   # statistical fallback — should essentially never happen
            sc = queries[b].astype(np.float64) @ candidates.astype(np.float64).T
            top = np.argpartition(-sc, k)[:k]
            top = top[np.lexsort((top, -sc[top]))]
            out_v[b] = sc[top].astype(np.float32)
            out_i[b] = top.astype(np.int32)
            continue
        out_v[b] = vals32[s:s + k]
        out_i[b] = ci[s:s + k].astype(np.int32)
    return out_v, out_i


def kernel(queries, candidates, k):
    import os
    from concourse import bass_utils

    k = int(k)
    queries = np.asarray(queries, np.float32)
    candidates = np.asarray(candidates, np.float32)
    in_maps, qq, cc, t_all = _prep_inputs(queries, candidates)
    nc = _get_nc()
    trace = os.environ.get("KNN_TRACE", "0") == "1"
    try:
        res = bass_utils.run_bass_kernel_spmd(nc, in_maps,
                                              core_ids=list(range(NCORES)),
                                              trace=trace)
    except ModuleNotFoundError:
        res = bass_utils.run_bass_kernel_spmd(nc, in_maps,
                                              core_ids=list(range(NCORES)))
    _CACHE["last_results"] = res
    core_outs = [(r["out_vals"], r["out_acc"]) for r in res.results]
    return _decode_and_merge(queries, candidates, core_outs, qq, cc, t_all, k)
